# revision 31
# baseline (speedup 1.0000x reference)
"""Cosine-similarity self-attention (Cos_Attn) on 8 Trainium2 NeuronCores.

Reference math (x: [C=512, W=64, H=64] fp32, N = W*H = 4096):
    q = x.reshape(C, N).T                  # [N, C]
    energy = q @ q.T                       # [N, N]
    cos    = energy / (|q_i| |q_j|)
    out    = softmax(cos, axis=-1)[None]   # [1, N, N]

v9 design - host-normalized fp8, query-major layout, PE warm-up,
hybrid row-sums (DVE bf16 add-tree for blocks 0-2, ACT accumulator for
the last block).

Host pre-normalizes the columns of x to unit L2 norm before the fp8
quantize, so on device cosine == dot product of fp8 unit vectors: no
Grams, no rsqrt chains, one ACT table load (pulled to t=0 by a dummy
exp). Inputs are host-permuted so every input DMA descriptor is a 4 KB
contiguous run.

Per core (own 512 query rows x all 4096 keys):
  - queries on PSUM partitions, keys free; output needs no transpose.
  - PE warm-up: a few dummy bf16 matmuls run during the input DMA so
    the Tensor engine leaves its cold pstate and real matmuls overlap
    their LDWEIGHTS immediately.
  - energy: per (128-query block, 2048-key half): 8 fp8 DoubleRow
    matmuls (K=256) into a 4-bank PSUM tile [128, 4, 512], double
    buffered (PE fills one while ACT drains the other).
  - exp: ONE activation per half ([128, 2048] f32 PSUM -> bf16 SBUF,
    scale=1/cq^2). ACT is the bottleneck engine (~2 us x 8 stream), so
    row sums stay off it where possible: blocks 0-2 sum via a bf16
    add-tree on the idle DVE (error ~1e-4, well under budget); the
    last block uses the ACT accumulator (accum_out) because its row
    sum sits on the exit critical path.
  - tail: reciprocal_approx_fast, all-bf16 per-partition scale (2x
    mode); early blocks' 1 MB out-DMAs overlap later blocks' compute;
    the last block's scale+DMA is split across the two free queues.
"""

import numpy as np

_NCORES = 8
_P = 128

# set by the test harness only; the grading path keeps these defaults
TRACE = False
TRACE_CORES = None
LAST_RESULT = None

_built = None  # (nc, C, N)

_CQ = 16.0     # host fp8 quantize scale for the normalized columns
_NWARM = 7     # PE pstate warm-up matmuls


def _build(C, N, QB):
    """Single-NEFF Bass/Tile program (SPMD: identical on all cores).

    Inputs:  x8 [128, C/128 * N]  fp8e4, host-permuted pair-major:
                 [p, pair(4), ko(4), 1024] with c = ko*128 + p
             xq [128, C/128 * QB] fp8e4, host-permuted: [p, ko(4), QB]
    Output:  out [QB, N] bf16 = this core's softmax rows.
    """
    from contextlib import ExitStack

    import concourse.tile as tile
    from concourse import bacc, mybir

    f32 = mybir.dt.float32
    bf16 = mybir.dt.bfloat16
    fp8 = mybir.dt.float8e4
    AF = mybir.ActivationFunctionType
    DR = mybir.MatmulPerfMode.DoubleRow

    P = _P
    KO = C // P              # contraction subtiles (4)
    NP = N // 1024           # key chunk pairs (4)
    QBLK = QB // P           # query blocks per core (4)
    ESC = 1.0 / (_CQ * _CQ)  # exp input scale: cos = energy / cq^2

    nc = bacc.Bacc("TRN2", target_bir_lowering=False, debug=False)
    x8_d = nc.dram_tensor("x8", [P, KO * N], fp8, kind="ExternalInput")
    xq_d = nc.dram_tensor("xq", [P, KO * QB], fp8, kind="ExternalInput")
    out_d = nc.dram_tensor("out", [QB, N], bf16, kind="ExternalOutput")

    x8_r = x8_d.ap().rearrange("p (c k x) -> p c k x", c=NP, k=KO)
    xq_r = xq_d.ap().rearrange("p (k x) -> p k x", k=KO)
    out_r = out_d.ap().rearrange("(qb p) (nk x) -> p qb nk x", p=P, x=512)

    with tile.TileContext(nc) as tc, ExitStack() as ctx:
        persist = ctx.enter_context(tc.tile_pool(name="persist", bufs=1))
        temps = ctx.enter_context(tc.tile_pool(name="temps", bufs=2))
        psum = ctx.enter_context(tc.tile_pool(name="psum", bufs=2, space="PSUM"))

        x8_sb = persist.tile([P, NP, KO, 1024], fp8)   # all keys, pair-major
        xq_sb = persist.tile([P, KO, QB], fp8)         # own query cols
        e_sb = persist.tile([P, QBLK, 2, 2048], bf16)  # exp(cos) rows
        rsum = persist.tile([P, QBLK, 2], f32)         # half row-sums
        rs = persist.tile([P, QBLK], f32)              # row sums
        rr = persist.tile([P, QBLK], f32)              # 1 / row sums
        warm = persist.tile([P, 1], f32)
        wdum = persist.tile([P, P], bf16)              # warm-up weights
        rdum = persist.tile([P, 512], bf16)            # warm-up rhs

        nc.vector.memset(warm[:], 0.0)
        nc.vector.memset(wdum[:], 0.0)
        nc.vector.memset(rdum[:], 0.0)

        # ---- input DMAs: pair0 + queries get the engines first; the
        # first query block consumes chunks in this arrival order ----
        nc.scalar.dma_start(x8_sb[:, 0], x8_r[:, 0])
        nc.sync.dma_start(xq_sb[:], xq_r[:])
        nc.sync.dma_start(x8_sb[:, 1], x8_r[:, 1])
        nc.gpsimd.dma_start(x8_sb[:, 2], x8_r[:, 2])
        nc.scalar.activation(warm[:], warm[:], AF.Exp)  # ACT table load now
        nc.scalar.dma_start(x8_sb[:, 3], x8_r[:, 3])

        # ---- PE pstate warm-up: keep the Tensor engine busy through
        # the input-DMA window so real matmuls run at full clock and
        # overlap their LDWEIGHTS from the first real instruction ----
        for _ in range(_NWARM):
            pd = psum.tile([P, 4, 512], f32, tag="pp", name="pp", bufs=2)
            nc.tensor.matmul(pd[:, 0, :], lhsT=wdum[:], rhs=rdum[:],
                             start=True, stop=True)

        # ---- first query block: chunk-granular, follows DMA arrival
        # so its exps start ~4 us earlier ----
        rq0 = persist.tile([P, 4], f32)
        qsl0 = slice(0, P)
        for c in range(4):
            pp = psum.tile([P, 4, 512], f32, tag="pp", name="pp", bufs=2)
            for j in range(2):
                cs = slice(j * 512, j * 512 + 512)
                for k2 in range(2):
                    k2s = slice(2 * k2, 2 * k2 + 2)
                    nc.tensor.matmul(
                        pp[:, j, :],
                        lhsT=xq_sb[:, k2s, qsl0],
                        rhs=x8_sb[:, c, k2s, cs],
                        start=(k2 == 0),
                        stop=(k2 == 1),
                        perf_mode=DR,
                    )
            eo = slice((c % 2) * 1024, (c % 2) * 1024 + 1024)
            nc.scalar.activation(
                e_sb[:, 0, c // 2, eo].rearrange("p (a x) -> p a x", a=2),
                pp[:, 0:2, :], AF.Exp, scale=ESC,
                accum_out=rq0[:, c:c + 1])
        nc.vector.tensor_reduce(rs[:, 0:1], rq0[:],
                                axis=mybir.AxisListType.X,
                                op=mybir.AluOpType.add)
        nc.vector.reciprocal_approx_fast(rr[:, 0:1], rs[:, 0:1])
        nc.vector.tensor_scalar_mul(e_sb[:, 0], e_sb[:, 0], rr[:, 0:1])
        nc.sync.dma_start(out_r[:, 0], e_sb[:, 0].rearrange(
            "p h (nk x) -> p (h nk) x", x=512))

        for qb in range(1, QBLK):
            qsl = slice(qb * P, (qb + 1) * P)
            last = qb == QBLK - 1
            for h in range(2):
                pp = psum.tile([P, 4, 512], f32, tag="pp", name="pp", bufs=2)
                for j in range(4):
                    cs = slice((j % 2) * 512, (j % 2) * 512 + 512)
                    for k2 in range(2):
                        k2s = slice(2 * k2, 2 * k2 + 2)
                        nc.tensor.matmul(
                            pp[:, j, :],
                            lhsT=xq_sb[:, k2s, qsl],
                            rhs=x8_sb[:, 2 * h + j // 2, k2s, cs],
                            start=(k2 == 0),
                            stop=(k2 == 1),
                            perf_mode=DR,
                        )
                nc.scalar.activation(
                    e_sb[:, qb, h].rearrange("p (a x) -> p a x", a=4), pp[:],
                    AF.Exp, scale=ESC,
                    accum_out=rsum[:, qb, h:h + 1])
            # ---- softmax denominator ----
            nc.vector.tensor_add(rs[:, qb:qb + 1], rsum[:, qb, 0:1],
                                 rsum[:, qb, 1:2])
            nc.vector.reciprocal_approx_fast(rr[:, qb:qb + 1], rs[:, qb:qb + 1])
            # ---- scale + stream out ----
            if not last:
                nc.vector.tensor_scalar_mul(e_sb[:, qb], e_sb[:, qb],
                                            rr[:, qb:qb + 1])
                nc.gpsimd.dma_start(out_r[:, qb], e_sb[:, qb].rearrange(
                    "p h (nk x) -> p (h nk) x", x=512))
            else:
                # last block: split scale + DMA across both queues
                for h in range(2):
                    nc.vector.tensor_scalar_mul(e_sb[:, qb, h], e_sb[:, qb, h],
                                                rr[:, qb:qb + 1])
                    eng = nc.sync if h == 0 else nc.scalar
                    eng.dma_start(out_r[:, qb, 4 * h:4 * h + 4],
                                  e_sb[:, qb, h].rearrange(
                                      "p (nk x) -> p nk x", x=512))

    nc.compile()
    return nc


def kernel(**inputs) -> np.ndarray:
    global _built, LAST_RESULT
    import ml_dtypes

    x = np.asarray(inputs["x"], dtype=np.float32)
    C, W, H = x.shape
    N = W * H
    QB = N // _NCORES
    x2 = x.reshape(C, N)

    if _built is None or _built[1:] != (C, N):
        _built = (_build(C, N, QB), C, N)
    nc = _built[0]

    from concourse import bass_utils

    # host preprocess: unit-normalize columns, fp8-quantize, and permute
    # into the device's per-partition layout (4 KB DMA runs).
    norms = np.sqrt((x2 * x2).sum(axis=0))
    x8 = (x2 * (_CQ / norms)[None, :]).astype(ml_dtypes.float8_e4m3fn)
    # x8[ko*128+p, c*1024+j] -> x8p[p, c, ko, j]
    x8p = np.ascontiguousarray(
        x8.reshape(C // _P, _P, N // 1024, 1024).transpose(1, 2, 0, 3)
    ).reshape(_P, -1)
    in_maps = []
    for i in range(_NCORES):
        xq = x8[:, i * QB:(i + 1) * QB]
        # xq[ko*128+p, q] -> xqp[p, ko, q]
        xqp = np.ascontiguousarray(
            xq.reshape(C // _P, _P, QB).transpose(1, 0, 2)).reshape(_P, -1)
        in_maps.append({"x8": x8p, "xq": xqp})

    kwargs = {}
    if TRACE:
        kwargs["trace"] = True
        if TRACE_CORES is not None:
            kwargs["trace_cores"] = list(TRACE_CORES)
    res = bass_utils.run_bass_kernel_spmd(
        nc, in_maps, core_ids=list(range(_NCORES)), **kwargs
    )
    LAST_RESULT = res
    out = np.empty((N, N), dtype=np.float32)
    for i in range(_NCORES):
        out[i * QB:(i + 1) * QB] = res.results[i]["out"].astype(np.float32)
    return out.reshape(1, N, N)


# revision 32
# speedup vs baseline: 1.0237x; 1.0237x over previous
"""Cosine-similarity self-attention (Cos_Attn) on 8 Trainium2 NeuronCores.

Reference math (x: [C=512, W=64, H=64] fp32, N = W*H = 4096):
    q = x.reshape(C, N).T                  # [N, C]
    energy = q @ q.T                       # [N, N]
    cos    = energy / (|q_i| |q_j|)
    out    = softmax(cos, axis=-1)[None]   # [1, N, N]

v9 design - host-normalized fp8, query-major layout, PE warm-up,
hybrid row-sums (DVE bf16 add-tree for blocks 0-2, ACT accumulator for
the last block).

Host pre-normalizes the columns of x to unit L2 norm before the fp8
quantize, so on device cosine == dot product of fp8 unit vectors: no
Grams, no rsqrt chains, one ACT table load (pulled to t=0 by a dummy
exp). Inputs are host-permuted so every input DMA descriptor is a 4 KB
contiguous run.

Per core (own 512 query rows x all 4096 keys):
  - queries on PSUM partitions, keys free; output needs no transpose.
  - PE warm-up: a few dummy bf16 matmuls run during the input DMA so
    the Tensor engine leaves its cold pstate and real matmuls overlap
    their LDWEIGHTS immediately.
  - energy: per (128-query block, 2048-key half): 8 fp8 DoubleRow
    matmuls (K=256) into a 4-bank PSUM tile [128, 4, 512], double
    buffered (PE fills one while ACT drains the other).
  - exp: ONE activation per half ([128, 2048] f32 PSUM -> bf16 SBUF,
    scale=1/cq^2). ACT is the bottleneck engine (~2 us x 8 stream), so
    row sums stay off it where possible: blocks 0-2 sum via a bf16
    add-tree on the idle DVE (error ~1e-4, well under budget); the
    last block uses the ACT accumulator (accum_out) because its row
    sum sits on the exit critical path.
  - tail: reciprocal_approx_fast, all-bf16 per-partition scale (2x
    mode); early blocks' 1 MB out-DMAs overlap later blocks' compute;
    the last block's scale+DMA is split across the two free queues.
"""

import numpy as np

_NCORES = 8
_P = 128

# set by the test harness only; the grading path keeps these defaults
TRACE = False
TRACE_CORES = None
LAST_RESULT = None

_built = None  # (nc, C, N)

_CQ = 16.0     # host fp8 quantize scale for the normalized columns
_NWARM = 7     # PE pstate warm-up matmuls


def _build(C, N, QB):
    """Single-NEFF Bass/Tile program (SPMD: identical on all cores).

    Inputs:  x8 [128, C/128 * N]  fp8e4, host-permuted pair-major:
                 [p, pair(4), ko(4), 1024] with c = ko*128 + p
             xq [128, C/128 * QB] fp8e4, host-permuted: [p, ko(4), QB]
    Output:  out [QB, N] bf16 = this core's softmax rows.
    """
    from contextlib import ExitStack

    import concourse.tile as tile
    from concourse import bacc, mybir

    f32 = mybir.dt.float32
    bf16 = mybir.dt.bfloat16
    fp8 = mybir.dt.float8e4
    AF = mybir.ActivationFunctionType
    DR = mybir.MatmulPerfMode.DoubleRow

    P = _P
    KO = C // P              # contraction subtiles (4)
    NP = N // 1024           # key chunk pairs (4)
    QBLK = QB // P           # query blocks per core (4)
    ESC = 1.0 / (_CQ * _CQ)  # exp input scale: cos = energy / cq^2

    nc = bacc.Bacc("TRN2", target_bir_lowering=False, debug=False)
    x8_d = nc.dram_tensor("x8", [P, KO * N], fp8, kind="ExternalInput")
    xq_d = nc.dram_tensor("xq", [P, KO * QB], fp8, kind="ExternalInput")
    out_d = nc.dram_tensor("out", [QB, N], bf16, kind="ExternalOutput")

    x8_r = x8_d.ap().rearrange("p (c k x) -> p c k x", c=NP, k=KO)
    xq_r = xq_d.ap().rearrange("p (k x) -> p k x", k=KO)
    out_r = out_d.ap().rearrange("(qb p) (nk x) -> p qb nk x", p=P, x=512)

    with tile.TileContext(nc) as tc, ExitStack() as ctx:
        persist = ctx.enter_context(tc.tile_pool(name="persist", bufs=1))
        temps = ctx.enter_context(tc.tile_pool(name="temps", bufs=2))
        psum = ctx.enter_context(tc.tile_pool(name="psum", bufs=2, space="PSUM"))

        x8_sb = persist.tile([P, NP, KO, 1024], fp8)   # all keys, pair-major
        xq_sb = persist.tile([P, KO, QB], fp8)         # own query cols
        e_sb = persist.tile([P, QBLK, 2, 2048], bf16)  # exp(cos) rows
        rsum = persist.tile([P, QBLK, 2], f32)         # half row-sums
        rs = persist.tile([P, QBLK], f32)              # row sums
        rr = persist.tile([P, QBLK], f32)              # 1 / row sums
        warm = persist.tile([P, 1], f32)
        wdum = persist.tile([P, P], bf16)              # warm-up weights
        rdum = persist.tile([P, 512], bf16)            # warm-up rhs

        nc.vector.memset(warm[:], 0.0)
        nc.vector.memset(wdum[:], 0.0)
        nc.vector.memset(rdum[:], 0.0)

        # ---- input DMAs: pair0 + queries get the engines first; the
        # first query block consumes chunks in this arrival order ----
        nc.scalar.dma_start(x8_sb[:, 0], x8_r[:, 0])
        nc.sync.dma_start(xq_sb[:], xq_r[:])
        nc.gpsimd.dma_start(x8_sb[:, 2], x8_r[:, 2])
        nc.scalar.activation(warm[:], warm[:], AF.Exp)  # ACT table load now
        nc.scalar.dma_start(x8_sb[:, 1], x8_r[:, 1])
        nc.scalar.dma_start(x8_sb[:, 3], x8_r[:, 3])

        # ---- PE pstate warm-up: keep the Tensor engine busy through
        # the input-DMA window so real matmuls run at full clock and
        # overlap their LDWEIGHTS from the first real instruction ----
        for _ in range(_NWARM):
            pd = psum.tile([P, 4, 512], f32, tag="pp", name="pp", bufs=2)
            nc.tensor.matmul(pd[:, 0, :], lhsT=wdum[:], rhs=rdum[:],
                             start=True, stop=True)

        # ---- first query block: chunk-granular, follows DMA arrival
        # so its exps start ~4 us earlier ----
        rq0 = persist.tile([P, 4], f32)
        qsl0 = slice(0, P)
        for c in range(4):
            pp = psum.tile([P, 4, 512], f32, tag="pp", name="pp", bufs=2)
            for j in range(2):
                cs = slice(j * 512, j * 512 + 512)
                for k2 in range(2):
                    k2s = slice(2 * k2, 2 * k2 + 2)
                    nc.tensor.matmul(
                        pp[:, j, :],
                        lhsT=xq_sb[:, k2s, qsl0],
                        rhs=x8_sb[:, c, k2s, cs],
                        start=(k2 == 0),
                        stop=(k2 == 1),
                        perf_mode=DR,
                    )
            eo = slice((c % 2) * 1024, (c % 2) * 1024 + 1024)
            nc.scalar.activation(
                e_sb[:, 0, c // 2, eo].rearrange("p (a x) -> p a x", a=2),
                pp[:, 0:2, :], AF.Exp, scale=ESC,
                accum_out=rq0[:, c:c + 1])
        nc.vector.tensor_reduce(rs[:, 0:1], rq0[:],
                                axis=mybir.AxisListType.X,
                                op=mybir.AluOpType.add)
        nc.vector.reciprocal_approx_fast(rr[:, 0:1], rs[:, 0:1])
        nc.vector.tensor_scalar_mul(e_sb[:, 0], e_sb[:, 0], rr[:, 0:1])
        nc.sync.dma_start(out_r[:, 0], e_sb[:, 0].rearrange(
            "p h (nk x) -> p (h nk) x", x=512))

        for qb in range(1, QBLK):
            qsl = slice(qb * P, (qb + 1) * P)
            last = qb == QBLK - 1
            for h in range(2):
                pp = psum.tile([P, 4, 512], f32, tag="pp", name="pp", bufs=2)
                for j in range(4):
                    cs = slice((j % 2) * 512, (j % 2) * 512 + 512)
                    for k2 in range(2):
                        k2s = slice(2 * k2, 2 * k2 + 2)
                        nc.tensor.matmul(
                            pp[:, j, :],
                            lhsT=xq_sb[:, k2s, qsl],
                            rhs=x8_sb[:, 2 * h + j // 2, k2s, cs],
                            start=(k2 == 0),
                            stop=(k2 == 1),
                            perf_mode=DR,
                        )
                nc.scalar.activation(
                    e_sb[:, qb, h].rearrange("p (a x) -> p a x", a=4), pp[:],
                    AF.Exp, scale=ESC,
                    accum_out=rsum[:, qb, h:h + 1])
            # ---- softmax denominator ----
            nc.vector.tensor_add(rs[:, qb:qb + 1], rsum[:, qb, 0:1],
                                 rsum[:, qb, 1:2])
            nc.vector.reciprocal_approx_fast(rr[:, qb:qb + 1], rs[:, qb:qb + 1])
            # ---- scale + stream out ----
            if not last:
                nc.vector.tensor_scalar_mul(e_sb[:, qb], e_sb[:, qb],
                                            rr[:, qb:qb + 1])
                nc.gpsimd.dma_start(out_r[:, qb], e_sb[:, qb].rearrange(
                    "p h (nk x) -> p (h nk) x", x=512))
            else:
                # last block: split scale + DMA across both queues
                for h in range(2):
                    nc.vector.tensor_scalar_mul(e_sb[:, qb, h], e_sb[:, qb, h],
                                                rr[:, qb:qb + 1])
                    eng = nc.sync if h == 0 else nc.scalar
                    eng.dma_start(out_r[:, qb, 4 * h:4 * h + 4],
                                  e_sb[:, qb, h].rearrange(
                                      "p (nk x) -> p nk x", x=512))

    nc.compile()
    return nc


def kernel(**inputs) -> np.ndarray:
    global _built, LAST_RESULT
    import ml_dtypes

    x = np.asarray(inputs["x"], dtype=np.float32)
    C, W, H = x.shape
    N = W * H
    QB = N // _NCORES
    x2 = x.reshape(C, N)

    if _built is None or _built[1:] != (C, N):
        _built = (_build(C, N, QB), C, N)
    nc = _built[0]

    from concourse import bass_utils

    # host preprocess: unit-normalize columns, fp8-quantize, and permute
    # into the device's per-partition layout (4 KB DMA runs).
    norms = np.sqrt((x2 * x2).sum(axis=0))
    x8 = (x2 * (_CQ / norms)[None, :]).astype(ml_dtypes.float8_e4m3fn)
    # x8[ko*128+p, c*1024+j] -> x8p[p, c, ko, j]
    x8p = np.ascontiguousarray(
        x8.reshape(C // _P, _P, N // 1024, 1024).transpose(1, 2, 0, 3)
    ).reshape(_P, -1)
    in_maps = []
    for i in range(_NCORES):
        xq = x8[:, i * QB:(i + 1) * QB]
        # xq[ko*128+p, q] -> xqp[p, ko, q]
        xqp = np.ascontiguousarray(
            xq.reshape(C // _P, _P, QB).transpose(1, 0, 2)).reshape(_P, -1)
        in_maps.append({"x8": x8p, "xq": xqp})

    kwargs = {}
    if TRACE:
        kwargs["trace"] = True
        if TRACE_CORES is not None:
            kwargs["trace_cores"] = list(TRACE_CORES)
    res = bass_utils.run_bass_kernel_spmd(
        nc, in_maps, core_ids=list(range(_NCORES)), **kwargs
    )
    LAST_RESULT = res
    out = np.empty((N, N), dtype=np.float32)
    for i in range(_NCORES):
        out[i * QB:(i + 1) * QB] = res.results[i]["out"].astype(np.float32)
    return out.reshape(1, N, N)


# revision 34
# speedup vs baseline: 1.0800x; 1.0550x over previous
"""Cosine-similarity self-attention (Cos_Attn) on 8 Trainium2 NeuronCores.

Reference math (x: [C=512, W=64, H=64] fp32, N = W*H = 4096):
    q = x.reshape(C, N).T                  # [N, C]
    energy = q @ q.T                       # [N, N]
    cos    = energy / (|q_i| |q_j|)
    out    = softmax(cos, axis=-1)[None]   # [1, N, N]

v16 design - host-normalized fp8, query-major layout, PE warm-up,
ACT-accumulated row sums, chunk-granular first block.

Host pre-normalizes the columns of x to unit L2 norm before the fp8
quantize, so on device cosine == dot product of fp8 unit vectors: no
Grams, no rsqrt chains, one ACT table load (pulled to t=0 by a dummy
exp). Inputs are host-permuted so every input DMA descriptor is a 4 KB
contiguous run.

Per core (own 512 query rows x all 4096 keys):
  - queries on PSUM partitions, keys free; output needs no transpose.
  - PE warm-up: a few dummy bf16 matmuls run during the input DMA so
    the Tensor engine leaves its cold pstate and real matmuls overlap
    their LDWEIGHTS immediately.
  - energy: per (128-query block, 2048-key half): 8 fp8 DoubleRow
    matmuls (K=256) into a 4-bank PSUM tile [128, 4, 512], double
    buffered (PE fills one while ACT drains the other).
  - head: the FIRST query block runs at chunk (1024-key) granularity
    so its first exp waits only for the first key pair (~14 us vs ~18)
    and follows the input-DMA arrivals; later blocks use 2048-key
    halves (lower ACT per-instruction overhead).
  - exp: ONE activation per half ([128, 2048] f32 PSUM -> bf16 SBUF,
    scale=1/cq^2) with accum_out producing the row-sum for ~180 ns;
    softmax denominators never touch the slow 1x-mode DVE reduce. ACT
    is the bottleneck engine (~2.2 us x 8 stream). DVE carries no exp
    work, so the tile scheduler interleaves the softmax scales (and
    their 1 MB out-DMAs) into the stream instead of deferring them.
  - tail: reciprocal_approx_fast, all-bf16 per-partition scale (2x
    mode); early blocks' 1 MB out-DMAs overlap later blocks' compute;
    the last block's scale+DMA is split across the two free queues.
"""

import numpy as np

_NCORES = 8
_P = 128

# set by the test harness only; the grading path keeps these defaults
TRACE = False
TRACE_CORES = None
LAST_RESULT = None

_built = None  # (nc, C, N)

_CQ = 16.0     # host fp8 quantize scale for the normalized columns
_NWARM = 11    # PE pstate warm-up matmuls


def _build(C, N, QB):
    """Single-NEFF Bass/Tile program (SPMD: identical on all cores).

    Inputs:  x8 [128, C/128 * N]  fp8e4, host-permuted pair-major:
                 [p, pair(4), ko(4), 1024] with c = ko*128 + p
             xq [128, C/128 * QB] fp8e4, host-permuted: [p, ko(4), QB]
    Output:  out [QB, N] bf16 = this core's softmax rows.
    """
    from contextlib import ExitStack

    import concourse.tile as tile
    from concourse import bacc, mybir

    f32 = mybir.dt.float32
    bf16 = mybir.dt.bfloat16
    fp8 = mybir.dt.float8e4
    AF = mybir.ActivationFunctionType
    DR = mybir.MatmulPerfMode.DoubleRow

    P = _P
    KO = C // P              # contraction subtiles (4)
    NP = N // 1024           # key chunk pairs (4)
    QBLK = QB // P           # query blocks per core (4)
    ESC = 1.0 / (_CQ * _CQ)  # exp input scale: cos = energy / cq^2

    nc = bacc.Bacc("TRN2", target_bir_lowering=False, debug=False)
    x8_d = nc.dram_tensor("x8", [P, KO * N], fp8, kind="ExternalInput")
    xq_d = nc.dram_tensor("xq", [P, KO * QB], fp8, kind="ExternalInput")
    out_d = nc.dram_tensor("out", [QB, N], bf16, kind="ExternalOutput")

    x8_r = x8_d.ap().rearrange("p (c k x) -> p c k x", c=NP, k=KO)
    xq_r = xq_d.ap().rearrange("p (k x) -> p k x", k=KO)
    out_r = out_d.ap().rearrange("(qb p) (nk x) -> p qb nk x", p=P, x=512)

    with tile.TileContext(nc) as tc, ExitStack() as ctx:
        persist = ctx.enter_context(tc.tile_pool(name="persist", bufs=1))
        temps = ctx.enter_context(tc.tile_pool(name="temps", bufs=2))
        psum = ctx.enter_context(tc.tile_pool(name="psum", bufs=2, space="PSUM"))

        x8_sb = persist.tile([P, NP, KO, 1024], fp8)   # all keys, pair-major
        xq_sb = persist.tile([P, KO, QB], fp8)         # own query cols
        e_sb = persist.tile([P, QBLK, 2, 2048], bf16)  # exp(cos) rows
        rsum = persist.tile([P, QBLK, 2], f32)         # half row-sums
        rs = persist.tile([P, QBLK], f32)              # row sums
        rr = persist.tile([P, QBLK], f32)              # 1 / row sums
        warm = persist.tile([P, 1], f32)
        wdum = persist.tile([P, P], bf16)              # warm-up weights
        rdum = persist.tile([P, 512], bf16)            # warm-up rhs

        nc.vector.memset(warm[:], 0.0)
        nc.vector.memset(wdum[:], 0.0)
        nc.vector.memset(rdum[:], 0.0)

        # ---- input DMAs: pair0 + queries get the engines first; the
        # first query block consumes chunks in this arrival order ----
        nc.scalar.dma_start(x8_sb[:, 0], x8_r[:, 0])
        nc.sync.dma_start(xq_sb[:], xq_r[:])
        nc.gpsimd.dma_start(x8_sb[:, 2], x8_r[:, 2])
        nc.scalar.activation(warm[:], warm[:], AF.Exp)  # ACT table load now
        nc.scalar.dma_start(x8_sb[:, 1], x8_r[:, 1])
        nc.scalar.dma_start(x8_sb[:, 3], x8_r[:, 3])

        # ---- PE pstate warm-up: keep the Tensor engine busy through
        # the input-DMA window so real matmuls run at full clock and
        # overlap their LDWEIGHTS from the first real instruction ----
        for _ in range(_NWARM):
            pd = psum.tile([P, 4, 512], f32, tag="pp", name="pp", bufs=2)
            nc.tensor.matmul(pd[:, 0, :], lhsT=wdum[:], rhs=rdum[:],
                             start=True, stop=True)

        # ---- first query block: chunk-granular, follows DMA arrival
        # so its exps start ~4 us earlier ----
        rq0 = persist.tile([P, 4], f32)
        qsl0 = slice(0, P)
        for c in range(4):
            pp = psum.tile([P, 4, 512], f32, tag="pp", name="pp", bufs=2)
            for j in range(2):
                cs = slice(j * 512, j * 512 + 512)
                for k2 in range(2):
                    k2s = slice(2 * k2, 2 * k2 + 2)
                    nc.tensor.matmul(
                        pp[:, j, :],
                        lhsT=xq_sb[:, k2s, qsl0],
                        rhs=x8_sb[:, c, k2s, cs],
                        start=(k2 == 0),
                        stop=(k2 == 1),
                        perf_mode=DR,
                    )
            eo = slice((c % 2) * 1024, (c % 2) * 1024 + 1024)
            nc.scalar.activation(
                e_sb[:, 0, c // 2, eo].rearrange("p (a x) -> p a x", a=2),
                pp[:, 0:2, :], AF.Exp, scale=ESC,
                accum_out=rq0[:, c:c + 1])
        nc.vector.tensor_reduce(rs[:, 0:1], rq0[:],
                                axis=mybir.AxisListType.X,
                                op=mybir.AluOpType.add)
        nc.vector.reciprocal_approx_fast(rr[:, 0:1], rs[:, 0:1])
        for h in range(2):
            nc.vector.tensor_scalar_mul(e_sb[:, 0, h], e_sb[:, 0, h],
                                        rr[:, 0:1])
            nc.sync.dma_start(out_r[:, 0, 4 * h:4 * h + 4],
                              e_sb[:, 0, h].rearrange(
                                  "p (nk x) -> p nk x", x=512))

        for qb in range(1, QBLK):
            qsl = slice(qb * P, (qb + 1) * P)
            last = qb == QBLK - 1
            for h in range(2):
                pp = psum.tile([P, 4, 512], f32, tag="pp", name="pp", bufs=2)
                for j in range(4):
                    cs = slice((j % 2) * 512, (j % 2) * 512 + 512)
                    for k2 in range(2):
                        k2s = slice(2 * k2, 2 * k2 + 2)
                        nc.tensor.matmul(
                            pp[:, j, :],
                            lhsT=xq_sb[:, k2s, qsl],
                            rhs=x8_sb[:, 2 * h + j // 2, k2s, cs],
                            start=(k2 == 0),
                            stop=(k2 == 1),
                            perf_mode=DR,
                        )
                nc.scalar.activation(
                    e_sb[:, qb, h].rearrange("p (a x) -> p a x", a=4), pp[:],
                    AF.Exp, scale=ESC,
                    accum_out=rsum[:, qb, h:h + 1])
            # ---- softmax denominator ----
            nc.vector.tensor_add(rs[:, qb:qb + 1], rsum[:, qb, 0:1],
                                 rsum[:, qb, 1:2])
            nc.vector.reciprocal_approx_fast(rr[:, qb:qb + 1], rs[:, qb:qb + 1])
            # ---- scale + stream out ----
            if not last:
                nc.vector.tensor_scalar_mul(e_sb[:, qb], e_sb[:, qb],
                                            rr[:, qb:qb + 1])
                nc.gpsimd.dma_start(out_r[:, qb], e_sb[:, qb].rearrange(
                    "p h (nk x) -> p (h nk) x", x=512))
            else:
                # last block: split scale + DMA across both queues
                for h in range(2):
                    nc.vector.tensor_scalar_mul(e_sb[:, qb, h], e_sb[:, qb, h],
                                                rr[:, qb:qb + 1])
                    eng = nc.sync if h == 0 else nc.scalar
                    eng.dma_start(out_r[:, qb, 4 * h:4 * h + 4],
                                  e_sb[:, qb, h].rearrange(
                                      "p (nk x) -> p nk x", x=512))

    nc.compile()
    return nc


def kernel(**inputs) -> np.ndarray:
    global _built, LAST_RESULT
    import ml_dtypes

    x = np.asarray(inputs["x"], dtype=np.float32)
    C, W, H = x.shape
    N = W * H
    QB = N // _NCORES
    x2 = x.reshape(C, N)

    if _built is None or _built[1:] != (C, N):
        _built = (_build(C, N, QB), C, N)
    nc = _built[0]

    from concourse import bass_utils

    # host preprocess: unit-normalize columns, fp8-quantize, and permute
    # into the device's per-partition layout (4 KB DMA runs).
    norms = np.sqrt((x2 * x2).sum(axis=0))
    x8 = (x2 * (_CQ / norms)[None, :]).astype(ml_dtypes.float8_e4m3fn)
    # x8[ko*128+p, c*1024+j] -> x8p[p, c, ko, j]
    x8p = np.ascontiguousarray(
        x8.reshape(C // _P, _P, N // 1024, 1024).transpose(1, 2, 0, 3)
    ).reshape(_P, -1)
    in_maps = []
    for i in range(_NCORES):
        xq = x8[:, i * QB:(i + 1) * QB]
        # xq[ko*128+p, q] -> xqp[p, ko, q]
        xqp = np.ascontiguousarray(
            xq.reshape(C // _P, _P, QB).transpose(1, 0, 2)).reshape(_P, -1)
        in_maps.append({"x8": x8p, "xq": xqp})

    kwargs = {}
    if TRACE:
        kwargs["trace"] = True
        if TRACE_CORES is not None:
            kwargs["trace_cores"] = list(TRACE_CORES)
    res = bass_utils.run_bass_kernel_spmd(
        nc, in_maps, core_ids=list(range(_NCORES)), **kwargs
    )
    LAST_RESULT = res
    out = np.empty((N, N), dtype=np.float32)
    for i in range(_NCORES):
        out[i * QB:(i + 1) * QB] = res.results[i]["out"].astype(np.float32)
    return out.reshape(1, N, N)
